# revision 1
# baseline (speedup 1.0000x reference)
"""DecayMaskedMultiHeadAttention on 8 trn2 NeuronCores (Bass/Tile SPMD).

Model: B=4, N=1024, DIM=1024, 16 heads x head_dim 64.
  q/k/v = x @ W.T + b ; scores = (q_h k_h^T)/8 * decaymask_h ;
  out = softmax(scores) v_h ; y = concat_h(out) @ wo.T + bo

Sharding (8 cores): 4 head-groups x 2 batch-groups.
  core c: head group g = c // 2 (heads 4g..4g+3), batch group p = c % 2
  (batches 2p, 2p+1). Each core computes a partial y for its 2 batches
  (its 4 heads' contribution through the out-projection); host sums the
  4 partials per batch group and adds the closed-form bias terms
  (bo + bv @ wo.T; attention rows sum to 1 so bv passes through).

On-core layout (all matmuls in float32r, fp32 PSUM accumulate):
  - host pre-transposes x -> xT [D, N] and decaymask -> maskT [k, q],
    plus weight slices, so no on-chip transposes are needed anywhere.
  - qT/kT [head_dim, tok] per 2-head stack; v natural [tok, dim] with a
    ones column appended per head (gives the softmax denominator as an
    extra output row of the attn@v matmul).
  - scores are computed transposed (scoresT [k, q] = kT.T @ qT), decay
    mask applied on DVE, exp on ACT; exp needs no max-subtraction
    here (scores are O(1) * mask in [0,1)).
  - attn@v: out_hT [65, q] = [v_h | 1].T @ expT, accumulated over k.
    Row 64 is the denominator; reciprocal + PE broadcast + DVE multiply
    normalizes rows 0..63 while evacuating PSUM.
  - out-projection: lhsT is exactly the stacked normalized out_hT, so
    partial y [tok, D] = lhsT.T @ woT accumulates over the 2 stacks.
  - 1/sqrt(head_dim) = 1/8 is folded into wq/bq on the host (exact).
"""

import numpy as np
import ml_dtypes

DIM = 1024
H = 16
HD = 64
B = 4
N = 1024
NCORES = 8
HPC = 4            # heads per core
BPC = 2            # batches per core
NSTACK = 2         # 2-head stacks per core
VBLK = HD + 1      # v block width per head (ones column appended)
VROW = HPC * VBLK  # v columns per 128-token chunk


_PROGRAM = None  # (nc, input_names) cache
LAST_RESULTS = None  # BassKernelResults from the most recent run (for test.py)


def _build_program(reps=1):
    import concourse.mybir as mybir
    import concourse.tile as tile
    from concourse import bacc

    f32 = mybir.dt.float32
    f32r = mybir.dt.float32r
    f16 = mybir.dt.float16
    AF = mybir.ActivationFunctionType

    nc = bacc.Bacc(
        "TRN2",
        target_bir_lowering=False,
        debug=False,
        num_devices=NCORES,
    )

    xT = nc.dram_tensor("xT", [BPC, DIM, N], f16, kind="ExternalInput").ap()
    maskT = nc.dram_tensor("maskT", [HPC, N, N], f16, kind="ExternalInput").ap()
    wqT = nc.dram_tensor("wqT", [DIM, HPC * HD], f16, kind="ExternalInput").ap()
    wkT = nc.dram_tensor("wkT", [DIM, HPC * HD], f16, kind="ExternalInput").ap()
    wvT = nc.dram_tensor("wvT", [DIM, HPC * HD], f16, kind="ExternalInput").ap()
    woT = nc.dram_tensor("woT", [HPC * HD, DIM], f16, kind="ExternalInput").ap()
    bqd = nc.dram_tensor("bq", [NSTACK, 128, 1], f32, kind="ExternalInput").ap()
    bkd = nc.dram_tensor("bk", [NSTACK, 128, 1], f32, kind="ExternalInput").ap()
    outp = nc.dram_tensor("outp", [BPC, N, DIM], f32, kind="ExternalOutput").ap()

    KC = DIM // 128  # 8 contraction chunks over D
    TC = N // 128    # 8 token chunks
    QH = N // 512    # 2 q halves (fp32 moving-operand limit is 512)

    with tile.TileContext(nc) as tc:
        with (
            tc.tile_pool(name="w", bufs=1) as wpool,
            tc.tile_pool(name="persist", bufs=1) as persist,
            tc.tile_pool(name="xt", bufs=16) as xpool,
            tc.tile_pool(name="maskp", bufs=16) as maskp,
            tc.tile_pool(name="expm", bufs=3) as expm_p,
            tc.tile_pool(name="expo", bufs=4) as expo_p,
            tc.tile_pool(name="ev", bufs=3) as ev_p,
            tc.tile_pool(name="small", bufs=2) as small_p,
            tc.tile_pool(name="psA", bufs=2, space="PSUM") as psA,
            tc.tile_pool(name="psS", bufs=2, space="PSUM") as psS,
            tc.tile_pool(name="psO", bufs=2, space="PSUM") as psO,
        ):
            # --- weights; actual DMAs are interleaved with x0 below ---
            wq_t = [wpool.tile([128, HPC * HD], f16, tag=f"wq{kc}", name=f"wq{kc}") for kc in range(KC)]
            wk_t = [wpool.tile([128, HPC * HD], f16, tag=f"wk{kc}", name=f"wk{kc}") for kc in range(KC)]
            wv_t = [wpool.tile([128, HPC * HD], f16, tag=f"wv{kc}", name=f"wv{kc}") for kc in range(KC)]
            bq_t = [wpool.tile([128, 1], f32, tag=f"bq{s}", name=f"bq{s}") for s in range(NSTACK)]
            bk_t = [wpool.tile([128, 1], f32, tag=f"bk{s}", name=f"bk{s}") for s in range(NSTACK)]

            # persistent activations
            qT = {}
            kT = {}
            ao = {}
            for b in range(BPC):
                for s in range(NSTACK):
                    qT[(b, s)] = persist.tile([128, N], f16, tag=f"qT{b}{s}", name=f"qT{b}{s}")
                    kT[(b, s)] = persist.tile([128, N], f16, tag=f"kT{b}{s}", name=f"kT{b}{s}")
                    ao[(b, s)] = persist.tile([128, N], f16, tag=f"ao{b}{s}", name=f"ao{b}{s}")
            vt = {b: persist.tile([128, TC * VROW], f16, tag=f"v{b}", name=f"v{b}") for b in range(BPC)}
            for b in range(BPC):
                # whole-tile fill; projection evacs overwrite the data
                # columns, leaving the per-head ones column = 1.0
                nc.vector.memset(vt[b][:], 1.0)

            xts = {}

            def load_x(b):
                for kc in range(KC):
                    t = xpool.tile([128, N], f16, tag="xts", name=f"xts{b}_{kc}")
                    nc.sync.dma_start(t[:], xT[b, kc * 128:(kc + 1) * 128, :])
                    xts[(b, kc)] = t

            def load_startup():
                """x0 + weights interleaved per kc so the first projection
                matmul can start after ~1 MB of DMA."""
                for kc in range(KC):
                    t = xpool.tile([128, N], f16, tag="xts", name=f"xts0_{kc}")
                    nc.sync.dma_start(t[:], xT[0, kc * 128:(kc + 1) * 128, :])
                    xts[(0, kc)] = t
                    nc.sync.dma_start(wq_t[kc][:], wqT[kc * 128:(kc + 1) * 128, :])
                    nc.sync.dma_start(wk_t[kc][:], wkT[kc * 128:(kc + 1) * 128, :])
                    nc.sync.dma_start(wv_t[kc][:], wvT[kc * 128:(kc + 1) * 128, :])
                    if kc == 0:
                        for s in range(NSTACK):
                            nc.sync.dma_start(bq_t[s][:], bqd[s])
                            nc.sync.dma_start(bk_t[s][:], bkd[s])
                for s in range(NSTACK):
                    nc.sync.dma_start(wo_t[s][:], woT[s * 128:(s + 1) * 128, :])

            def qk_group(b, wt, bt, dst, s, qh):
                ps = psA.tile([128, 512], f32, tag="big",
                              name=f"pj{b}{s}{qh}{'q' if wt is wq_t else 'k'}")
                for kc in range(KC):
                    nc.tensor.matmul(
                        ps[:],
                        lhsT=wt[kc][:, s * 128:(s + 1) * 128],
                        rhs=xts[(b, kc)][:, qh * 512:(qh + 1) * 512],
                        start=(kc == 0),
                        stop=(kc == KC - 1),
                    )
                nc.scalar.activation(
                    dst[(b, s)][:, qh * 512:(qh + 1) * 512],
                    ps[:],
                    AF.Identity,
                    bias=bt[s][:],
                    scale=1.0,
                )

            def v_group(b, tci):
                ps = psA.tile([128, 512], f32, tag="big", name=f"pjv{b}{tci}")
                for kc in range(KC):
                    nc.tensor.matmul(
                        ps[:, 0:HPC * HD],
                        lhsT=xts[(b, kc)][:, tci * 128:(tci + 1) * 128],
                        rhs=wv_t[kc][:],
                        start=(kc == 0),
                        stop=(kc == KC - 1),
                    )
                dst = vt[b][:, tci * VROW:(tci + 1) * VROW].rearrange(
                    "p (h e) -> p h e", e=VBLK
                )[:, :, 0:HD]
                nc.scalar.activation(
                    dst, ps[:, 0:HPC * HD].rearrange("p (h e) -> p h e", e=HD),
                    AF.Copy,
                )

            def qk_groups(b, s):
                out = []
                for wt, bt, dst in ((wq_t, bq_t, qT), (wk_t, bk_t, kT)):
                    for qh in range(QH):
                        out.append(lambda b=b, wt=wt, bt=bt, dst=dst, s=s, qh=qh:
                                   qk_group(b, wt, bt, dst, s, qh))
                return out

            mask_t = {}

            def preload_mask(h):
                for kc in range(TC):
                    mt = maskp.tile([128, N], f16, tag="mask", name=f"mask{h}_{kc}")
                    nc.gpsimd.dma_start(mt[:], maskT[h, kc * 128:(kc + 1) * 128, :])
                    mask_t[(h, kc)] = mt

            def attn(h, b, fillers=None, per_slot=1):
                """Attention for head h (local), batch b. b=0 loads mask tiles.
                Pops up to per_slot PE filler groups per kc iteration."""
                s, hh = h // 2, h % 2
                op = {}
                for qh in range(QH):
                    op[qh] = psO.tile([VBLK, 512], f32, tag=f"ov{qh}", name=f"ov{h}{b}{qh}")
                if b == 1 and h + 2 < HPC:
                    preload_mask(h + 2)
                for kc in range(TC):
                    if fillers:
                        for _ in range(per_slot):
                            if fillers:
                                fillers.pop(0)()
                    mt = mask_t[(h, kc)]
                    em = expm_p.tile([128, N], f32, tag="expm", name=f"expm{h}{kc}{b}")
                    for qh in range(QH):
                        sc = psS.tile([128, 512], f32, tag="sc", name=f"sc{h}{kc}{b}{qh}")
                        nc.tensor.matmul(
                            sc[:],
                            lhsT=kT[(b, s)][hh * HD:(hh + 1) * HD,
                                            kc * 128:(kc + 1) * 128],
                            rhs=qT[(b, s)][hh * HD:(hh + 1) * HD,
                                           qh * 512:(qh + 1) * 512],
                            start=True,
                            stop=True,
                        )
                        nc.vector.tensor_mul(
                            em[:, qh * 512:(qh + 1) * 512],
                            sc[:],
                            mt[:, qh * 512:(qh + 1) * 512],
                        )
                    eo = expo_p.tile([128, N], f16, tag="expo", name=f"expo{h}{kc}{b}")
                    nc.scalar.activation(eo[:], em[:], AF.Exp)
                    vblk = vt[b][:, kc * VROW + h * VBLK:kc * VROW + (h + 1) * VBLK]
                    for qh in range(QH):
                        nc.tensor.matmul(
                            op[qh][:],
                            lhsT=vblk,
                            rhs=eo[:, qh * 512:(qh + 1) * 512],
                            start=(kc == 0),
                            stop=(kc == TC - 1),
                        )
                for qh in range(QH):
                    rc = small_p.tile([1, 512], f32, tag="rec", name=f"rec{h}{b}{qh}")
                    nc.vector.reciprocal(rc[:], op[qh][HD:VBLK, :])
                    bcs = small_p.tile([HD, 512], f32, tag="bcs", name=f"bcs{h}{b}{qh}")
                    nc.gpsimd.partition_broadcast(bcs[:], rc[:])
                    nc.vector.tensor_mul(
                        ao[(b, s)][hh * HD:(hh + 1) * HD, qh * 512:(qh + 1) * 512],
                        op[qh][0:HD, :],
                        bcs[:],
                    )

            def outproj_group(b, tci, dh):
                        po = psA.tile([128, 512], f32, tag="big", name=f"po{b}{tci}{dh}")
                        for s in range(NSTACK):
                            nc.tensor.matmul(
                                po[:],
                                lhsT=ao[(b, s)][:, tci * 128:(tci + 1) * 128],
                                rhs=wo_t[s][:, dh * 512:(dh + 1) * 512],
                                start=(s == 0),
                                stop=(s == NSTACK - 1),
                            )
                        ot = ev_p.tile([128, 512], f32, tag="ot", name=f"ot{b}{tci}{dh}")
                        if tci % 2 == 0:
                            nc.vector.tensor_copy(ot[:], po[:])
                        else:
                            nc.scalar.copy(ot[:], po[:])
                        nc.sync.dma_start(
                            outp[b, tci * 128:(tci + 1) * 128,
                                 dh * 512:(dh + 1) * 512],
                            ot[:],
                        )

            wo_t = [wpool.tile([128, DIM], f16, tag=f"wo{s}", name=f"wo{s}") for s in range(NSTACK)]

            # Software-pipelined emission. Projections for stack s1 of batch 0
            # plus all of batch 1's projections ride as PE "fillers" inside
            # the attention kc loops; out-projection of batch 0 hides under
            # the final attention phase of batch 1. reps>1 repeats the whole
            # computation in-NEFF (timing amplification only).
            for _rep in range(reps):
                mask_t.clear()
                xts.clear()
                preload_mask(0)
                preload_mask(1)
                load_startup()
                for g in qk_groups(0, 0):
                    g()
                for tci in range(TC):
                    v_group(0, tci)
                load_x(1)
                fillers = (qk_groups(0, 1) + qk_groups(1, 0)
                           + [lambda tci=tci: v_group(1, tci) for tci in range(TC)]
                           + qk_groups(1, 1))
                attn(0, 0, fillers, per_slot=2)
                attn(0, 1, fillers, per_slot=1)
                attn(1, 0, fillers, per_slot=1)
                attn(1, 1, fillers, per_slot=1)
                attn(2, 0)
                attn(2, 1)
                attn(3, 0)
                fillers2 = [lambda b=0, tci=tci, dh=dh: outproj_group(0, tci, dh)
                            for tci in range(TC) for dh in range(QH)]
                attn(3, 1, fillers2, per_slot=2)
                for f in fillers2:
                    f()
                for tci in range(TC):
                    for dh in range(QH):
                        outproj_group(1, tci, dh)

    nc.compile()
    return nc


def _get_program():
    global _PROGRAM
    if _PROGRAM is None:
        _PROGRAM = _build_program()
    return _PROGRAM


def kernel(x, decaymask, wq, bq, wk, bk, wv, bv, wo, bo):
    from concourse.bass_utils import run_bass_kernel_spmd

    global LAST_RESULTS

    x = np.ascontiguousarray(np.asarray(x, dtype=np.float32))
    decaymask = np.ascontiguousarray(np.asarray(decaymask, dtype=np.float32))
    wq = np.asarray(wq, dtype=np.float32)
    bq = np.asarray(bq, dtype=np.float32)
    wk = np.asarray(wk, dtype=np.float32)
    bk = np.asarray(bk, dtype=np.float32)
    wv = np.asarray(wv, dtype=np.float32)
    bv = np.asarray(bv, dtype=np.float32)
    wo = np.asarray(wo, dtype=np.float32)
    bo = np.asarray(bo, dtype=np.float32)

    nc = _get_program()

    in_maps = []
    for c in range(NCORES):
        g, p = c // 2, c % 2
        rows = slice(g * HPC * HD, (g + 1) * HPC * HD)
        xT_c = np.ascontiguousarray(
            x[p * BPC:(p + 1) * BPC].transpose(0, 2, 1)
        ).astype(np.float16)  # [BPC, D, N]
        maskT_c = np.ascontiguousarray(
            decaymask[g * HPC:(g + 1) * HPC].transpose(0, 2, 1)
        ).astype(np.float16)  # [HPC, k, q]
        # fold 1/sqrt(HD) = 1/8 (exact) into wq/bq
        wqT_c = (np.ascontiguousarray(wq[rows, :].T) * np.float32(0.125)).astype(np.float16)
        wkT_c = np.ascontiguousarray(wk[rows, :].T).astype(np.float16)
        wvT_c = np.ascontiguousarray(wv[rows, :].T).astype(np.float16)
        woT_c = np.ascontiguousarray(wo[:, rows].T).astype(np.float16)
        bq_c = (bq[rows] * np.float32(0.125)).reshape(NSTACK, 128, 1)
        bk_c = bk[rows].reshape(NSTACK, 128, 1).copy()
        in_maps.append({
            "xT": xT_c,
            "maskT": maskT_c,
            "wqT": wqT_c,
            "wkT": wkT_c,
            "wvT": wvT_c,
            "woT": woT_c,
            "bq": np.ascontiguousarray(bq_c),
            "bk": bk_c,
        })

    res = run_bass_kernel_spmd(nc, in_maps, list(range(NCORES)))
    LAST_RESULTS = res

    out = np.zeros((B, N, DIM), dtype=np.float32)
    for c in range(NCORES):
        g, p = c // 2, c % 2
        out[p * BPC:(p + 1) * BPC] += res.results[c]["outp"]
    out += (bo + bv @ wo.T)[None, None, :]
    return out



# revision 4
# speedup vs baseline: 1.4842x; 1.4842x over previous
"""DecayMaskedMultiHeadAttention on 8 trn2 NeuronCores (Bass/Tile SPMD).

Model: B=4, N=1024, DIM=1024, 16 heads x head_dim 64.
  q/k/v = x @ W.T + b ; scores = (q_h k_h^T)/8 * decaymask_h ;
  out = softmax(scores) v_h ; y = concat_h(out) @ wo.T + bo

Sharding (8 cores): 4 head-groups x 2 batch-groups.
  core c: head group g = c // 2 (heads 4g..4g+3), batch group p = c % 2
  (batches 2p, 2p+1). Each core computes a partial y for its 2 batches;
  host sums the 4 partials per batch group and adds bo + bv @ wo.T
  (attention rows sum to 1 so bv passes through exactly).

v2 (vs the f16 baseline):
  - fp8(e4m3) x / wq / wk / wv / v with DoubleRow matmuls (contraction
    pairs of 128-subtiles) for the q/k/v projections and attn@v.
  - scores stay f16 (qT/kT f16); exp output is fp8 with a constant
    bias of -1.5 inside exp (cancels between numerator & denominator,
    keeps exp values < 240 = fp8e4 max).
  - wide ops: scores PSUM tile [128,1024] (one DVE mask-mul per kc),
    exp over [128, 2, 1024] (FD 2048, one ACT op per kc pair).
  - reciprocal_approx_fast for softmax denominators (5x vs reciprocal).
  - schedule: all 4 heads of batch 0 first (b1 projections ride as PE
    fillers), then batch 1 (outproj b0 as fillers); all 4 mask heads
    stay resident in SBUF so each is DMA'd once.
  - output f16 (host accumulates partials in f32).
"""

import numpy as np
import ml_dtypes

DIM = 1024
H = 16
HD = 64
B = 4
N = 1024
NCORES = 8
HPC = 4            # heads per core
BPC = 2            # batches per core
NSTACK = 2         # 2-head stacks per core
VBLK = 68          # v block per head: 64 data + ones col(64) + 3 pad(=1)
VROW = HPC * VBLK  # v columns per 128-token chunk
KC = DIM // 128    # 8 contraction chunks over D
TC = N // 128      # 8 token chunks
QH = N // 512      # 2 q halves (PSUM bank = 512 fp32)

_PROGRAM = None
LAST_RESULTS = None  # BassKernelResults from the most recent run (for test.py)


def _build_program():
    import concourse.mybir as mybir
    import concourse.tile as tile
    from concourse import bacc

    f32 = mybir.dt.float32
    f16 = mybir.dt.float16
    fp8 = mybir.dt.float8e4
    AF = mybir.ActivationFunctionType
    DR = mybir.MatmulPerfMode.DoubleRow

    nc = bacc.Bacc(
        "TRN2",
        target_bir_lowering=False,
        debug=False,
        num_devices=NCORES,
    )

    xT = nc.dram_tensor("xT", [BPC, DIM, N], fp8, kind="ExternalInput").ap()
    maskT = nc.dram_tensor("maskT", [HPC, N, N], f16, kind="ExternalInput").ap()
    wqT = nc.dram_tensor("wqT", [DIM, HPC * HD], fp8, kind="ExternalInput").ap()
    wkT = nc.dram_tensor("wkT", [DIM, HPC * HD], fp8, kind="ExternalInput").ap()
    wvT = nc.dram_tensor("wvT", [DIM, HPC * HD], fp8, kind="ExternalInput").ap()
    woT = nc.dram_tensor("woT", [HPC * HD, DIM], f16, kind="ExternalInput").ap()
    bqd = nc.dram_tensor("bq", [NSTACK, 128, 1], f32, kind="ExternalInput").ap()
    bkd = nc.dram_tensor("bk", [NSTACK, 128, 1], f32, kind="ExternalInput").ap()
    outp = nc.dram_tensor("outp", [BPC, N, DIM], f16, kind="ExternalOutput").ap()

    with tile.TileContext(nc) as tc:
        with (
            tc.tile_pool(name="w", bufs=1) as wpool,
            tc.tile_pool(name="persist", bufs=1) as persist,
            tc.tile_pool(name="maskp", bufs=32) as maskp,
            tc.tile_pool(name="expm", bufs=3) as expm_p,
            tc.tile_pool(name="expo", bufs=3) as expo_p,
            tc.tile_pool(name="ev", bufs=3) as ev_p,
            tc.tile_pool(name="small", bufs=3) as small_p,
            tc.tile_pool(name="psA", bufs=2, space="PSUM") as psA,
            tc.tile_pool(name="psS", bufs=2, space="PSUM") as psS,
            tc.tile_pool(name="psO", bufs=1, space="PSUM") as psO,
        ):
            # --- persistent weights / activations ---
            wq_t = wpool.tile([128, KC, HPC * HD], fp8, tag="wq", name="wq")
            wk_t = wpool.tile([128, KC, HPC * HD], fp8, tag="wk", name="wk")
            wv_t = wpool.tile([128, KC, HPC * HD], fp8, tag="wv", name="wv")
            wo_t = [wpool.tile([128, DIM], f16, tag=f"wo{s}", name=f"wo{s}")
                    for s in range(NSTACK)]
            bq_t = [wpool.tile([128, 1], f32, tag=f"bq{s}", name=f"bq{s}") for s in range(NSTACK)]
            bk_t = [wpool.tile([128, 1], f32, tag=f"bk{s}", name=f"bk{s}") for s in range(NSTACK)]
            bneg = wpool.tile([128, 1], f32, tag="bneg", name="bneg")
            nc.gpsimd.memset(bneg[:], -1.5)

            qT = {}
            kT = {}
            ao = {}
            for b in range(BPC):
                for s in range(NSTACK):
                    qT[(b, s)] = persist.tile([128, N], f16, tag=f"qT{b}{s}", name=f"qT{b}{s}")
                    kT[(b, s)] = persist.tile([128, N], f16, tag=f"kT{b}{s}", name=f"kT{b}{s}")
                    ao[(b, s)] = persist.tile([128, N], f16, tag=f"ao{b}{s}", name=f"ao{b}{s}")
            xts = {b: persist.tile([128, KC, N], fp8, tag=f"x{b}", name=f"x{b}")
                   for b in range(BPC)}
            vt = {b: persist.tile([128, TC * VROW], fp8, tag=f"v{b}", name=f"v{b}")
                  for b in range(BPC)}
            for b in range(BPC):
                # whole-tile fill; projection evacs overwrite the data
                # columns, leaving ones col(64) + pad cols(65..67) = 1.0
                nc.gpsimd.memset(vt[b][:], 1.0)

            def load_x(b):
                for kc in range(KC):
                    nc.sync.dma_start(xts[b][:, kc, :], xT[b, kc * 128:(kc + 1) * 128, :])

            def load_startup():
                """x0 + weights interleaved per kc so the first projection
                matmul can start after a fraction of the DMA."""
                for kc in range(KC):
                    nc.sync.dma_start(xts[0][:, kc, :], xT[0, kc * 128:(kc + 1) * 128, :])
                    nc.sync.dma_start(wq_t[:, kc, :], wqT[kc * 128:(kc + 1) * 128, :])
                    nc.sync.dma_start(wk_t[:, kc, :], wkT[kc * 128:(kc + 1) * 128, :])
                    nc.sync.dma_start(wv_t[:, kc, :], wvT[kc * 128:(kc + 1) * 128, :])
                    if kc == 0:
                        for s in range(NSTACK):
                            nc.sync.dma_start(bq_t[s][:], bqd[s])
                            nc.sync.dma_start(bk_t[s][:], bkd[s])
                for s in range(NSTACK):
                    nc.sync.dma_start(wo_t[s][:], woT[s * 128:(s + 1) * 128, :])

            def qk_dr(b, wt, bt, dst, s, qh):
                ps = psA.tile([128, 512], f32, tag="big",
                              name=f"pj{b}{s}{qh}{'q' if wt is wq_t else 'k'}")
                for kp in range(KC // 2):
                    nc.tensor.matmul(
                        ps[:],
                        lhsT=wt[:, 2 * kp:2 * kp + 2, s * 128:(s + 1) * 128],
                        rhs=xts[b][:, 2 * kp:2 * kp + 2, qh * 512:(qh + 1) * 512],
                        start=(kp == 0),
                        stop=(kp == KC // 2 - 1),
                        perf_mode=DR,
                    )
                nc.scalar.activation(
                    dst[(b, s)][:, qh * 512:(qh + 1) * 512],
                    ps[:],
                    AF.Identity,
                    bias=bt[s][:],
                    scale=1.0,
                )

            def v_dr(b, tp):
                """v projection for token chunks 2tp, 2tp+1 (one evac)."""
                ps = psA.tile([128, 512], f32, tag="big", name=f"pjv{b}{tp}")
                for half in range(2):
                    tci = 2 * tp + half
                    for kp in range(KC // 2):
                        nc.tensor.matmul(
                            ps[:, half * 256:(half + 1) * 256],
                            lhsT=xts[b][:, 2 * kp:2 * kp + 2, tci * 128:(tci + 1) * 128],
                            rhs=wv_t[:, 2 * kp:2 * kp + 2, :],
                            start=(kp == 0),
                            stop=(kp == KC // 2 - 1),
                            perf_mode=DR,
                        )
                dst = vt[b].rearrange("p (c h e) -> p c h e", h=HPC, e=VBLK)[
                    :, 2 * tp:2 * tp + 2, :, 0:HD]
                src = ps[:].rearrange("p (c h e) -> p c h e", c=2, e=HD)
                nc.scalar.activation(dst, src, AF.Copy)

            def qk_groups(b):
                out = []
                for s in range(NSTACK):
                    for wt, bt, dst in ((wq_t, bq_t, qT), (wk_t, bk_t, kT)):
                        for qh in range(QH):
                            out.append(lambda b=b, wt=wt, bt=bt, dst=dst, s=s, qh=qh:
                                       qk_dr(b, wt, bt, dst, s, qh))
                return out

            mask_t = {}

            def preload_mask(h):
                for kc in range(TC):
                    mt = maskp.tile([128, N], f16, tag="mask", name=f"mask{h}_{kc}")
                    nc.gpsimd.dma_start(mt[:], maskT[h, kc * 128:(kc + 1) * 128, :])
                    mask_t[(h, kc)] = mt

            v3d = {b: vt[b].rearrange("p (c v) -> p c v", v=VROW) for b in range(BPC)}

            def attn(h, b, fillers=None, per_slot=1):
                """Attention for head h (local), batch b."""
                s, hh = h // 2, h % 2
                op = {}
                for qh in range(QH):
                    op[qh] = psO.tile([VBLK, 512], f32, tag=f"ov{qh}", name=f"ov{h}{b}{qh}")
                if b == 0 and h < 2:
                    preload_mask(h + 2)
                for kcp in range(TC // 2):
                    em = expm_p.tile([128, 2, N], f16, tag="expm", name=f"expm{h}{kcp}{b}")
                    eo = expo_p.tile([128, 2, N], fp8, tag="expo", name=f"expo{h}{kcp}{b}")
                    for j in range(2):
                        kc = 2 * kcp + j
                        if fillers:
                            for _ in range(per_slot):
                                if fillers:
                                    fillers.pop(0)()
                        sc = psS.tile([128, 1024], f32, tag="sc", name=f"sc{h}{kc}{b}")
                        for qh in range(QH):
                            nc.tensor.matmul(
                                sc[:, qh * 512:(qh + 1) * 512],
                                lhsT=kT[(b, s)][hh * HD:(hh + 1) * HD,
                                                kc * 128:(kc + 1) * 128],
                                rhs=qT[(b, s)][hh * HD:(hh + 1) * HD,
                                               qh * 512:(qh + 1) * 512],
                                start=True,
                                stop=True,
                            )
                        nc.vector.tensor_mul(em[:, j, :], sc[:], mask_t[(h, kc)][:])
                    nc.scalar.activation(eo[:], em[:], AF.Exp, bias=bneg[:])
                    vblk = v3d[b][:, 2 * kcp:2 * kcp + 2,
                                  h * VBLK:(h + 1) * VBLK]
                    for qh in range(QH):
                        nc.tensor.matmul(
                            op[qh][:],
                            lhsT=vblk,
                            rhs=eo[:, :, qh * 512:(qh + 1) * 512],
                            start=(kcp == 0),
                            stop=(kcp == TC // 2 - 1),
                            perf_mode=DR,
                        )
                for qh in range(QH):
                    rc = small_p.tile([1, 512], f32, tag="rec", name=f"rec{h}{b}{qh}")
                    nc.vector.reciprocal_approx_fast(rc[:], op[qh][HD:HD + 1, :])
                    bcs = small_p.tile([HD, 512], f32, tag="bcs", name=f"bcs{h}{b}{qh}")
                    nc.gpsimd.partition_broadcast(bcs[:], rc[:])
                    nc.vector.tensor_mul(
                        ao[(b, s)][hh * HD:(hh + 1) * HD, qh * 512:(qh + 1) * 512],
                        op[qh][0:HD, :],
                        bcs[:],
                    )

            def outproj_group(b, tci, dh):
                po = psA.tile([128, 512], f32, tag="big", name=f"po{b}{tci}{dh}")
                for s in range(NSTACK):
                    nc.tensor.matmul(
                        po[:],
                        lhsT=ao[(b, s)][:, tci * 128:(tci + 1) * 128],
                        rhs=wo_t[s][:, dh * 512:(dh + 1) * 512],
                        start=(s == 0),
                        stop=(s == NSTACK - 1),
                    )
                ot = ev_p.tile([128, 512], f16, tag="ot", name=f"ot{b}{tci}{dh}")
                nc.scalar.activation(ot[:], po[:], AF.Copy)
                nc.sync.dma_start(
                    outp[b, tci * 128:(tci + 1) * 128,
                         dh * 512:(dh + 1) * 512],
                    ot[:],
                )

            # --- emission schedule ---
            load_startup()
            preload_mask(0)
            preload_mask(1)
            for g in qk_groups(0):
                g()
            for tp in range(TC // 2):
                v_dr(0, tp)
            load_x(1)
            fillers = qk_groups(1) + [lambda tp=tp: v_dr(1, tp)
                                      for tp in range(TC // 2)]
            attn(0, 0, fillers)
            attn(1, 0, fillers)
            attn(2, 0, fillers)
            attn(3, 0, fillers)
            for f in fillers:
                f()
            fillers2 = [lambda tci=tci, dh=dh: outproj_group(0, tci, dh)
                        for tci in range(TC) for dh in range(QH)]
            attn(0, 1, fillers2)
            attn(1, 1, fillers2)
            attn(2, 1, fillers2)
            attn(3, 1, fillers2)
            for f in fillers2:
                f()
            for tci in range(TC):
                for dh in range(QH):
                    outproj_group(1, tci, dh)

    nc.compile()
    return nc


def _get_program():
    global _PROGRAM
    if _PROGRAM is None:
        _PROGRAM = _build_program()
    return _PROGRAM


def kernel(x, decaymask, wq, bq, wk, bk, wv, bv, wo, bo):
    from concourse.bass_utils import run_bass_kernel_spmd

    global LAST_RESULTS

    fp8 = ml_dtypes.float8_e4m3

    x = np.ascontiguousarray(np.asarray(x, dtype=np.float32))
    decaymask = np.ascontiguousarray(np.asarray(decaymask, dtype=np.float32))
    wq = np.asarray(wq, dtype=np.float32)
    bq = np.asarray(bq, dtype=np.float32)
    wk = np.asarray(wk, dtype=np.float32)
    bk = np.asarray(bk, dtype=np.float32)
    wv = np.asarray(wv, dtype=np.float32)
    bv = np.asarray(bv, dtype=np.float32)
    wo = np.asarray(wo, dtype=np.float32)
    bo = np.asarray(bo, dtype=np.float32)

    nc = _get_program()

    in_maps = []
    for c in range(NCORES):
        g, p = c // 2, c % 2
        rows = slice(g * HPC * HD, (g + 1) * HPC * HD)
        xT_c = np.ascontiguousarray(
            x[p * BPC:(p + 1) * BPC].transpose(0, 2, 1)
        ).astype(fp8)  # [BPC, D, N]
        maskT_c = np.ascontiguousarray(
            decaymask[g * HPC:(g + 1) * HPC].transpose(0, 2, 1)
        ).astype(np.float16)  # [HPC, k, q]
        # fold 1/sqrt(HD) = 1/8 (exact) into wq/bq
        wqT_c = (np.ascontiguousarray(wq[rows, :].T) * np.float32(0.125)).astype(fp8)
        wkT_c = np.ascontiguousarray(wk[rows, :].T).astype(fp8)
        wvT_c = np.ascontiguousarray(wv[rows, :].T).astype(fp8)
        woT_c = np.ascontiguousarray(wo[:, rows].T).astype(np.float16)
        bq_c = (bq[rows] * np.float32(0.125)).reshape(NSTACK, 128, 1)
        bk_c = bk[rows].reshape(NSTACK, 128, 1).copy()
        in_maps.append({
            "xT": xT_c,
            "maskT": maskT_c,
            "wqT": wqT_c,
            "wkT": wkT_c,
            "wvT": wvT_c,
            "woT": woT_c,
            "bq": np.ascontiguousarray(bq_c),
            "bk": bk_c,
        })

    res = run_bass_kernel_spmd(nc, in_maps, list(range(NCORES)))
    LAST_RESULTS = res

    out = np.zeros((B, N, DIM), dtype=np.float32)
    for c in range(NCORES):
        g, p = c // 2, c % 2
        out[p * BPC:(p + 1) * BPC] += res.results[c]["outp"].astype(np.float32)
    out += (bo + bv @ wo.T)[None, None, :]
    return out
